# revision 7
# baseline (speedup 1.0000x reference)
"""2-layer GCN (GCNConv -> ReLU -> GCNConv -> ReLU) on 8 Trainium2 NeuronCores.

Math (per layer, following PyG GCNConv):
    out = D^-1/2 (A + I) D^-1/2 (x @ W) + b
We exploit associativity so the sparse aggregation always runs on 128 features:
    layer1: h1 = relu( (A_norm @ x) @ W1 + b1 )          (aggregate first)
    layer2: out = relu( A_norm @ (h1 @ W2) + b2 )        (transform first)
Self-loops are appended as ordinary edges; per-edge weight norm_e =
dinv[src]*dinv[dst] makes the weighted segment-sum exactly A_norm.

Sharding: nodes are split into 8 contiguous shards of OWN=ceil(N/1024)*128
rows; edges are partitioned by destination owner so each core's segment-sum
is local.  Each core gathers source rows from a full replica of x (layer 1)
and from an AllGather'ed t2 = h1@W2 (layer 2).

Gathers use the gpsimd dma_gather custom op (one instruction fetches
thousands of rows).  Its indices are int16, so the source table is viewed in
4 banks of 25088 rows; edges are bucketed per (dst-block, src-bank), each
bucket padded to a multiple of 128 with dummy index 0 / weight 0.

Device algorithm per 128-node destination block (chunks of 128 edges):
    - per-bank dma_gather fetches the chunk rows   -> M [128e, C, 128f]
    - DVE builds S^T[e, i] = (iota[i] == dst_rel[e]) * norm[e] per chunk
    - PE accumulates  psum[f, i] += M_c^T @ S_c^T  over the block's chunks
giving the aggregated block transposed ([feat, dst]), which feeds the dense
transforms without any transpose; PE transpose mode is used only to emit
row-major t2 / output tiles.
"""

import math

import numpy as np

P = 128
NCORES = 8
D_IN, D_HID, D_OUT = 128, 256, 128
GB = 4  # dst blocks per dense group (psum free dim = GB*128 <= 512)
BANK = 25088  # int16-addressable rows per gather-table view
NBANK = 4

_CACHE: dict = {}


def _build(n_x_rows, OWN, n_blocks, Kbj, CH, timing_variant=False):
    import concourse.bacc as bacc
    import concourse.mybir as mybir
    import concourse.tile as tile
    from concourse.masks import make_identity

    FP = mybir.dt.float32
    I16 = mybir.dt.int16
    AF = mybir.ActivationFunctionType
    ALU = mybir.AluOpType

    n_cat_rows = OWN * NCORES
    # chunk bookkeeping (shared across cores)
    # global chunk index for (b, j, k): blocks major, banks inner, k last
    chunk_base = np.zeros((n_blocks, NBANK), np.int64)
    acc = 0
    for b in range(n_blocks):
        for j in range(NBANK):
            chunk_base[b, j] = acc
            acc += Kbj[b][j]
    assert acc == CH
    # per-bank cumulative chunk counts (for idx-column offsets)
    cumK = np.zeros((NBANK, n_blocks + 1), np.int64)
    for j in range(NBANK):
        for b in range(n_blocks):
            cumK[j, b + 1] = cumK[j, b] + Kbj[b][j]
    Cj = [int(cumK[j, n_blocks]) for j in range(NBANK)]

    NSWQ = 4  # rotate gathers over SWDGE queues
    MAXC = 8  # chunks per dma_gather call (1024 descs = SWDGE ring capacity)
    nc = bacc.Bacc("TRN2", debug=False, num_devices=NCORES, num_swdge_queues=NSWQ)

    x_d = nc.dram_tensor("x", [n_x_rows, D_IN], FP, kind="ExternalInput")
    w1_d = nc.dram_tensor("w1", [D_IN, D_HID], FP, kind="ExternalInput")
    w2_d = nc.dram_tensor("w2", [D_HID, D_OUT], FP, kind="ExternalInput")
    b1_d = nc.dram_tensor("b1h", [P, 2], FP, kind="ExternalInput")
    b2_d = nc.dram_tensor("b2c", [P, 1], FP, kind="ExternalInput")
    iota_d = nc.dram_tensor("iota", [P, P], FP, kind="ExternalInput")
    dr_d = nc.dram_tensor("dst_rel", [P, CH], FP, kind="ExternalInput")
    nm_d = nc.dram_tensor("norm", [P, CH], FP, kind="ExternalInput")
    idx_ds = [
        nc.dram_tensor(f"idx{j}", [P, max(Cj[j], 1) * 8], I16, kind="ExternalInput")
        for j in range(NBANK)
    ]
    out_d = nc.dram_tensor("out", [OWN, D_OUT], FP, kind="ExternalOutput")
    t2_own = nc.dram_tensor("t2_own", [OWN, D_OUT], FP)
    t2_cat = nc.dram_tensor("t2_cat", [n_cat_rows, D_OUT], FP, addr_space="Shared")

    ngroups = (n_blocks + GB - 1) // GB

    with tile.TileContext(nc) as tc:
        with (
            tc.tile_pool(name="const", bufs=1) as constp,
            tc.tile_pool(name="mp", bufs=2) as mp,
            tc.tile_pool(name="sp", bufs=4) as sp,
            tc.tile_pool(name="aggs", bufs=2) as aggs,
            tc.tile_pool(name="hs", bufs=2) as hs,
            tc.tile_pool(name="t2s", bufs=2) as t2s,
            tc.tile_pool(name="tps", bufs=3) as tps,
            tc.tile_pool(name="zs", bufs=2) as zs,
            tc.tile_pool(name="aggp", bufs=3, space="PSUM") as aggp,
            tc.tile_pool(name="dps", bufs=2, space="PSUM") as dps,
            tc.tile_pool(name="tpp", bufs=3, space="PSUM") as tpp,
        ):
            iota_t = constp.tile([P, P], FP, tag="iota")
            nc.sync.dma_start(iota_t[:, :], iota_d[:, :])
            ident = constp.tile([P, P], FP, tag="ident")
            make_identity(nc, ident[:, :])
            w1a = constp.tile([P, P], FP, tag="w1a")
            nc.sync.dma_start(w1a[:, :], w1_d[:, 0:P])
            w1b = constp.tile([P, P], FP, tag="w1b")
            nc.sync.dma_start(w1b[:, :], w1_d[:, P : 2 * P])
            w2a = constp.tile([P, P], FP, tag="w2a")
            nc.sync.dma_start(w2a[:, :], w2_d[0:P, :])
            w2b = constp.tile([P, P], FP, tag="w2b")
            nc.sync.dma_start(w2b[:, :], w2_d[P : 2 * P, :])
            b1t = constp.tile([P, 2], FP, tag="b1")
            nc.sync.dma_start(b1t[:, :], b1_d[:, :])
            b2t = constp.tile([P, 1], FP, tag="b2")
            nc.sync.dma_start(b2t[:, :], b2_d[:, :])
            sdst = constp.tile([P, CH], FP, tag="sdst")
            nc.sync.dma_start(sdst[:, :], dr_d[:, :])
            snorm = constp.tile([P, CH], FP, tag="snorm")
            nc.sync.dma_start(snorm[:, :], nm_d[:, :])
            idx_ts = []
            for j in range(NBANK):
                it = constp.tile([P, max(Cj[j], 1) * 8], I16, tag=f"idx{j}", name=f"idxt{j}")
                nc.sync.dma_start(it[:, :], idx_ds[j][:, :])
                idx_ts.append(it)

            qrot = [0]

            def gather_group(src_dram, n_rows, blocks):
                """dma_gather calls (<= MAXC chunks each) per source bank."""
                mts = {}
                for j in range(NBANK):
                    cg = int(cumK[j, blocks[-1] + 1] - cumK[j, blocks[0]])
                    if cg == 0:
                        continue
                    mt = mp.tile([P, cg, P], FP, tag=f"m{j}", name=f"mt{j}")
                    s0 = int(cumK[j, blocks[0]]) * 8
                    lo = j * BANK
                    hi = min((j + 1) * BANK, n_rows)
                    for c0 in range(0, cg, MAXC):
                        cc = min(MAXC, cg - c0)
                        nc.gpsimd.dma_gather(
                            out_ap=mt[:, c0 : c0 + cc, :],
                            in_ap=src_dram[lo:hi, :],
                            idxs_ap=idx_ts[j][:, s0 + c0 * 8 : s0 + (c0 + cc) * 8],
                            num_idxs=cc * P,
                            num_idxs_reg=cc * P,
                            elem_size=P,
                            queue_num=qrot[0] % NSWQ,
                        )
                        qrot[0] += 1
                    mts[j] = mt
                return mts

            def spmm_block(mts, blocks, b):
                """Weighted segment-sum of block b -> PSUM [128 feat, 128 dst]."""
                ps = aggp.tile([P, P], FP, tag="agg")
                nchunks = sum(Kbj[b])
                ci = 0
                for j in range(NBANK):
                    if Kbj[b][j] == 0:
                        continue
                    loc0 = int(cumK[j, b] - cumK[j, blocks[0]])
                    for k in range(Kbj[b][j]):
                        ch = int(chunk_base[b, j]) + k
                        s = sp.tile([P, P], FP, tag="s")
                        nc.vector.tensor_scalar(
                            out=s[:, :],
                            in0=iota_t[:, :],
                            scalar1=sdst[:, ch : ch + 1],
                            scalar2=snorm[:, ch : ch + 1],
                            op0=ALU.is_equal,
                            op1=ALU.mult,
                        )
                        nc.tensor.matmul(
                            out=ps[:, :],
                            lhsT=mts[j][:, loc0 + k, :],
                            rhs=s[:, :],
                            start=(ci == 0),
                            stop=(ci == nchunks - 1),
                        )
                        ci += 1
                return ps

            # ---------------- layer 1 + dense transform to t2 ----------------
            for g in range(ngroups):
                blocks = list(range(g * GB, min((g + 1) * GB, n_blocks)))
                nb = len(blocks)
                W = nb * P
                mts = gather_group(x_d, n_x_rows, blocks)
                aggsb = aggs.tile([P, GB * P], FP, tag="aggsb")
                for i, b in enumerate(blocks):
                    ps = spmm_block(mts, blocks, b)
                    nc.scalar.activation(aggsb[:, i * P : (i + 1) * P], ps[:, :], AF.Copy)
                h1 = []
                for h in range(2):
                    hp = dps.tile([P, GB * P], FP, tag="big")
                    nc.tensor.matmul(
                        out=hp[:, :W],
                        lhsT=(w1a, w1b)[h][:, :],
                        rhs=aggsb[:, :W],
                        start=True,
                        stop=True,
                    )
                    hb = hs.tile([P, GB * P], FP, tag=f"h1{h}")
                    nc.scalar.activation(hb[:, :W], hp[:, :W], AF.Relu, bias=b1t[:, h : h + 1])
                    h1.append(hb)
                tp_ = dps.tile([P, GB * P], FP, tag="big")
                nc.tensor.matmul(out=tp_[:, :W], lhsT=w2a[:, :], rhs=h1[0][:, :W], start=True, stop=False)
                nc.tensor.matmul(out=tp_[:, :W], lhsT=w2b[:, :], rhs=h1[1][:, :W], start=False, stop=True)
                t2b = t2s.tile([P, GB * P], FP, tag="t2b")
                nc.scalar.activation(t2b[:, :W], tp_[:, :W], AF.Copy)
                for i, b in enumerate(blocks):
                    tpps = tpp.tile([P, P], FP, tag="tp")
                    nc.tensor.transpose(out=tpps[:, :], in_=t2b[:, i * P : (i + 1) * P], identity=ident[:, :])
                    tsb = tps.tile([P, P], FP, tag="tsb")
                    nc.scalar.activation(tsb[:, :], tpps[:, :], AF.Copy)
                    r0 = b * P
                    nc.sync.dma_start(t2_own[r0 : r0 + P, :], tsb[:, :])

            # ---------------- exchange t2 shards ----------------
            if timing_variant:
                # single-core timing build: stand-in DMA for the collective
                # (its real cost is added from the measured-latency table)
                nc.sync.dma_start(t2_cat[0:OWN, :], t2_own[:, :])
            else:
                nc.gpsimd.collective_compute(
                    "AllGather",
                    ALU.bypass,
                    replica_groups=[list(range(NCORES))],
                    ins=[t2_own[:, :]],
                    outs=[t2_cat[:, :]],
                )

            # ---------------- layer 2 ----------------
            for g in range(ngroups):
                blocks = list(range(g * GB, min((g + 1) * GB, n_blocks)))
                mts = gather_group(t2_cat, n_cat_rows, blocks)
                for b in blocks:
                    ps = spmm_block(mts, blocks, b)
                    z = zs.tile([P, P], FP, tag="z")
                    nc.scalar.activation(z[:, :], ps[:, :], AF.Relu, bias=b2t[:, 0:1])
                    tpps = tpp.tile([P, P], FP, tag="tp")
                    nc.tensor.transpose(out=tpps[:, :], in_=z[:, :], identity=ident[:, :])
                    tsb = tps.tile([P, P], FP, tag="tsb")
                    nc.scalar.activation(tsb[:, :], tpps[:, :], AF.Copy)
                    r0 = b * P
                    nc.sync.dma_start(out_d[r0 : r0 + P, :], tsb[:, :])

    nc.compile()
    return nc


def _preprocess(x, edge_index, W1, b1, W2, b2):
    N = x.shape[0]
    OWN = int(math.ceil(N / (NCORES * P))) * P
    n_blocks = OWN // P
    NBLK = NCORES * n_blocks

    src = np.asarray(edge_index[0], np.int64)
    dst = np.asarray(edge_index[1], np.int64)
    loops = np.arange(N, dtype=np.int64)
    src_all = np.concatenate([src, loops])
    dst_all = np.concatenate([dst, loops])

    deg = np.bincount(dst_all, minlength=N).astype(np.float64)
    dinv = np.where(deg > 0, 1.0 / np.sqrt(np.maximum(deg, 1.0)), 0.0).astype(np.float32)
    norm_all = dinv[src_all] * dinv[dst_all]

    gblk = dst_all // P
    bank = src_all // BANK
    cell = gblk * NBANK + bank
    order = np.argsort(cell, kind="stable")
    s_src = src_all[order]
    s_dst = dst_all[order]
    s_norm = norm_all[order].astype(np.float32)
    s_cell = cell[order]
    s_bank = s_src // BANK

    counts = np.bincount(s_cell, minlength=NBLK * NBANK)
    percell = counts.reshape(NCORES, n_blocks, NBANK)
    Kbj = np.ceil(percell.max(axis=0) / P).astype(np.int64)  # [n_blocks, NBANK]
    Kbj[:, 0] = np.maximum(Kbj[:, 0], 1)  # every block gets >= 1 chunk
    caps = Kbj * P

    # slot offsets within the per-core padded edge stream, (block major, bank)
    cell_off = np.concatenate(([0], np.cumsum(caps.ravel())))[:-1].reshape(n_blocks, NBANK)
    TOT = int(caps.sum())
    CH = int(Kbj.sum())

    starts = np.concatenate(([0], np.cumsum(counts)))[:-1]
    pos = np.arange(s_dst.size) - starts[s_cell]
    core = (gblk[order] // n_blocks).astype(np.int64)
    lblk = (gblk[order] % n_blocks).astype(np.int64)
    slot = cell_off[lblk, s_bank] + pos

    arr_rel = np.zeros((NCORES, TOT), np.int16)
    arr_dst = np.zeros((NCORES, TOT), np.float32)
    arr_nrm = np.zeros((NCORES, TOT), np.float32)
    arr_rel[core, slot] = (s_src - s_bank * BANK).astype(np.int16)
    arr_dst[core, slot] = (s_dst % P).astype(np.float32)
    arr_nrm[core, slot] = s_norm

    # chunk-major staging [cores, 128, CH]
    stage_dst = np.ascontiguousarray(arr_dst.reshape(NCORES, CH, P).transpose(0, 2, 1))
    stage_nrm = np.ascontiguousarray(arr_nrm.reshape(NCORES, CH, P).transpose(0, 2, 1))

    # per-bank int16 index streams, 16-partition wrapped, replicated to 128 rows
    chunk_bank = np.repeat(
        np.tile(np.arange(NBANK), n_blocks), Kbj.ravel()
    )  # [CH] bank of each chunk
    rel3 = arr_rel.reshape(NCORES, CH, P)
    idx_stages = []
    for j in range(NBANK):
        selj = chunk_bank == j
        cj = int(selj.sum())
        if cj == 0:
            idx_stages.append(np.zeros((NCORES, P, 8), np.int16))
            continue
        flat = rel3[:, selj, :].reshape(NCORES, cj * P)
        w = flat.reshape(NCORES, cj * 8, 16).transpose(0, 2, 1)  # [cores, 16, cj*8]
        idx_stages.append(np.ascontiguousarray(np.tile(w, (1, 8, 1))))

    xf = np.ascontiguousarray(np.asarray(x, np.float32))
    w1 = np.ascontiguousarray(np.asarray(W1, np.float32))
    w2 = np.ascontiguousarray(np.asarray(W2, np.float32))
    b1h = np.ascontiguousarray(np.asarray(b1, np.float32).reshape(2, P).T)
    b2c = np.ascontiguousarray(np.asarray(b2, np.float32).reshape(P, 1))
    iota = np.ascontiguousarray(np.tile(np.arange(P, dtype=np.float32), (P, 1)))

    in_maps = []
    for c in range(NCORES):
        m = {
            "x": xf,
            "w1": w1,
            "w2": w2,
            "b1h": b1h,
            "b2c": b2c,
            "iota": iota,
            "dst_rel": stage_dst[c],
            "norm": stage_nrm[c],
        }
        for j in range(NBANK):
            m[f"idx{j}"] = idx_stages[j][c]
        in_maps.append(m)
    return in_maps, N, OWN, n_blocks, [list(map(int, r)) for r in Kbj], CH


def run(x, edge_index, W1, b1, W2, b2, trace=False):
    from concourse.bass_utils import run_bass_kernel_spmd

    in_maps, N, OWN, n_blocks, Kbj, CH = _preprocess(x, edge_index, W1, b1, W2, b2)
    key = (N, OWN, n_blocks, CH, tuple(tuple(r) for r in Kbj))
    nc = _CACHE.get(key)
    if nc is None:
        nc = _build(N, OWN, n_blocks, Kbj, CH)
        _CACHE[key] = nc

    res = run_bass_kernel_spmd(nc, in_maps, core_ids=list(range(NCORES)), trace=trace)
    out = np.concatenate([res.results[c]["out"] for c in range(NCORES)], axis=0)[:N]
    return np.ascontiguousarray(out.astype(np.float32)), res


def kernel(x, edge_index, W1, b1, W2, b2):
    out, _ = run(x, edge_index, W1, b1, W2, b2, trace=False)
    return out


def estimate_time_ns(np_inputs):
    """Cost-model (TimelineSim) per-core time estimate + AllGather table cost."""
    from concourse.timeline_sim import TimelineSim

    in_maps, N, OWN, n_blocks, Kbj, CH = _preprocess(**np_inputs)
    key = ("timing", N, OWN, n_blocks, CH, tuple(tuple(r) for r in Kbj))
    nc = _CACHE.get(key)
    if nc is None:
        nc = _build(N, OWN, n_blocks, Kbj, CH, timing_variant=True)
        _CACHE[key] = nc
    ts = TimelineSim(nc)
    t = ts.simulate()
    AG_NS = 35000.0  # 8-core AllGather @ ~6.4MB/rank (measured-latency table)
    return t + AG_NS


# revision 14
# speedup vs baseline: 1.1735x; 1.1735x over previous
"""2-layer GCN (GCNConv -> ReLU -> GCNConv -> ReLU) on 8 Trainium2 NeuronCores.

Math (per layer, following PyG GCNConv):
    out = D^-1/2 (A + I) D^-1/2 (x @ W) + b
We exploit associativity so the sparse aggregation always runs on 128 features:
    layer1: h1 = relu( (A_norm @ x) @ W1 + b1 )          (aggregate first)
    layer2: out = relu( A_norm @ (h1 @ W2) + b2 )        (transform first)
Self-loops are appended as ordinary edges; per-edge weight norm_e =
dinv[src]*dinv[dst] makes the weighted segment-sum exactly A_norm.

Sharding: nodes are split into 8 contiguous shards of OWN=ceil(N/1024)*128
rows; edges are partitioned by destination owner so each core's segment-sum
is local.  Each core gathers source rows from a full replica of x (layer 1)
and from an AllGather'ed t2 = h1@W2 (layer 2).

Gathers use the gpsimd dma_gather custom op (one instruction fetches
thousands of rows).  Its indices are int16, so the source table is viewed in
4 banks of 25088 rows; edges are bucketed per (dst-block, src-bank), each
bucket padded to a multiple of 128 with dummy index 0 / weight 0.

Device algorithm per 128-node destination block (chunks of 128 edges):
    - per-bank dma_gather fetches the chunk rows   -> M [128e, C, 128f]
    - DVE builds S^T[e, i] = (iota[i] == dst_rel[e]) * norm[e] per chunk
    - PE accumulates  psum[f, i] += M_c^T @ S_c^T  over the block's chunks
giving the aggregated block transposed ([feat, dst]), which feeds the dense
transforms without any transpose; PE transpose mode is used only to emit
row-major t2 / output tiles.
"""

import math

import numpy as np

P = 128
NCORES = 8
D_IN, D_HID, D_OUT = 128, 256, 128
GB = 4  # dst blocks per dense group (psum free dim = GB*128 <= 512)
BANK = 25088  # int16-addressable rows per gather-table view
NBANK = 4

_CACHE: dict = {}


def _build(n_x_rows, OWN, n_blocks, Kbj, CH, timing_variant=False):
    import concourse.bacc as bacc
    import concourse.mybir as mybir
    import concourse.tile as tile
    from concourse.masks import make_identity

    FP = mybir.dt.float32
    I16 = mybir.dt.int16
    AF = mybir.ActivationFunctionType
    ALU = mybir.AluOpType

    n_cat_rows = OWN * NCORES
    # chunk bookkeeping (shared across cores)
    # per block: [self chunk, bank0 chunks..., bank3 chunks...]
    self_chunk = np.zeros(n_blocks, np.int64)
    chunk_base = np.zeros((n_blocks, NBANK), np.int64)
    acc = 0
    for b in range(n_blocks):
        self_chunk[b] = acc
        acc += 1
        for j in range(NBANK):
            chunk_base[b, j] = acc
            acc += Kbj[b][j]
    assert acc == CH
    # per-bank cumulative chunk counts (for idx-column offsets)
    cumK = np.zeros((NBANK, n_blocks + 1), np.int64)
    for j in range(NBANK):
        for b in range(n_blocks):
            cumK[j, b + 1] = cumK[j, b] + Kbj[b][j]
    Cj = [int(cumK[j, n_blocks]) for j in range(NBANK)]

    NSWQ = 4  # rotate gathers over SWDGE queues
    MAXC = 8  # chunks per dma_gather call (1024 descs = SWDGE ring capacity)
    nc = bacc.Bacc("TRN2", debug=False, num_devices=NCORES, num_swdge_queues=NSWQ)

    x_d = nc.dram_tensor("x", [n_x_rows, D_IN], FP, kind="ExternalInput")
    xo_d = nc.dram_tensor("x_own", [OWN, D_IN], FP, kind="ExternalInput")
    w1_d = nc.dram_tensor("w1", [D_IN, D_HID], FP, kind="ExternalInput")
    w2_d = nc.dram_tensor("w2", [D_HID, D_OUT], FP, kind="ExternalInput")
    b1_d = nc.dram_tensor("b1h", [P, 2], FP, kind="ExternalInput")
    b2_d = nc.dram_tensor("b2c", [P, 1], FP, kind="ExternalInput")
    iota_d = nc.dram_tensor("iota", [P, P], FP, kind="ExternalInput")
    dr_d = nc.dram_tensor("dst_rel", [P, CH], FP, kind="ExternalInput")
    nm_d = nc.dram_tensor("norm", [P, CH], FP, kind="ExternalInput")
    idx_ds = [
        nc.dram_tensor(f"idx{j}", [P, max(Cj[j], 1) * 8], I16, kind="ExternalInput")
        for j in range(NBANK)
    ]
    out_d = nc.dram_tensor("out", [OWN, D_OUT], FP, kind="ExternalOutput")
    t2_own = nc.dram_tensor("t2_own", [OWN, D_OUT], FP)
    t2_cat = nc.dram_tensor("t2_cat", [n_cat_rows, D_OUT], FP, addr_space="Shared")

    ngroups = (n_blocks + GB - 1) // GB

    with tile.TileContext(nc) as tc:
        with (
            tc.tile_pool(name="const", bufs=1) as constp,
            tc.tile_pool(name="mp", bufs=2) as mp,
            tc.tile_pool(name="sp", bufs=4) as sp,
            tc.tile_pool(name="aggs", bufs=2) as aggs,
            tc.tile_pool(name="hs", bufs=2) as hs,
            tc.tile_pool(name="t2s", bufs=2) as t2s,
            tc.tile_pool(name="tps", bufs=3) as tps,
            tc.tile_pool(name="zs", bufs=2) as zs,
            tc.tile_pool(name="aggp", bufs=3, space="PSUM") as aggp,
            tc.tile_pool(name="dps", bufs=2, space="PSUM") as dps,
            tc.tile_pool(name="tpp", bufs=3, space="PSUM") as tpp,
        ):
            iota_t = constp.tile([P, P], FP, tag="iota")
            nc.sync.dma_start(iota_t[:, :], iota_d[:, :])
            ident = constp.tile([P, P], FP, tag="ident")
            make_identity(nc, ident[:, :])
            w1a = constp.tile([P, P], FP, tag="w1a")
            nc.sync.dma_start(w1a[:, :], w1_d[:, 0:P])
            w1b = constp.tile([P, P], FP, tag="w1b")
            nc.sync.dma_start(w1b[:, :], w1_d[:, P : 2 * P])
            w2a = constp.tile([P, P], FP, tag="w2a")
            nc.sync.dma_start(w2a[:, :], w2_d[0:P, :])
            w2b = constp.tile([P, P], FP, tag="w2b")
            nc.sync.dma_start(w2b[:, :], w2_d[P : 2 * P, :])
            b1t = constp.tile([P, 2], FP, tag="b1")
            nc.sync.dma_start(b1t[:, :], b1_d[:, :])
            b2t = constp.tile([P, 1], FP, tag="b2")
            nc.sync.dma_start(b2t[:, :], b2_d[:, :])
            sdst = constp.tile([P, CH], FP, tag="sdst")
            nc.sync.dma_start(sdst[:, :], dr_d[:, :])
            snorm = constp.tile([P, CH], FP, tag="snorm")
            nc.sync.dma_start(snorm[:, :], nm_d[:, :])
            idx_ts = []
            for j in range(NBANK):
                it = constp.tile([P, max(Cj[j], 1) * 8], I16, tag=f"idx{j}", name=f"idxt{j}")
                nc.sync.dma_start(it[:, :], idx_ds[j][:, :])
                idx_ts.append(it)

            qrot = [0]

            def gather_group(src_dram, n_rows, blocks):
                """dma_gather calls (<= MAXC chunks each) per source bank."""
                mts = {}
                for j in range(NBANK):
                    cg = int(cumK[j, blocks[-1] + 1] - cumK[j, blocks[0]])
                    if cg == 0:
                        continue
                    mt = mp.tile([P, cg, P], FP, tag=f"m{j}", name=f"mt{j}")
                    s0 = int(cumK[j, blocks[0]]) * 8
                    lo = j * BANK
                    hi = min((j + 1) * BANK, n_rows)
                    for c0 in range(0, cg, MAXC):
                        cc = min(MAXC, cg - c0)
                        nc.gpsimd.dma_gather(
                            out_ap=mt[:, c0 : c0 + cc, :],
                            in_ap=src_dram[lo:hi, :],
                            idxs_ap=idx_ts[j][:, s0 + c0 * 8 : s0 + (c0 + cc) * 8],
                            num_idxs=cc * P,
                            num_idxs_reg=cc * P,
                            elem_size=P,
                            queue_num=qrot[0] % NSWQ,
                        )
                        qrot[0] += 1
                    mts[j] = mt
                return mts

            def spmm_block(mts, blocks, b, own_dram):
                """Weighted segment-sum of block b -> PSUM [128 feat, 128 dst].

                Chunk 0 is the self-loop chunk: its "gathered" rows are the
                block's own contiguous rows (plain DMA), S = diag(dinv^2)."""
                ps = aggp.tile([P, P], FP, tag="agg")
                nchunks = 1 + sum(Kbj[b])

                def s_tile(ch):
                    s = sp.tile([P, P], FP, tag="s", name="s")
                    nc.vector.tensor_scalar(
                        out=s[:, :],
                        in0=iota_t[:, :],
                        scalar1=sdst[:, ch : ch + 1],
                        scalar2=snorm[:, ch : ch + 1],
                        op0=ALU.is_equal,
                        op1=ALU.mult,
                    )
                    return s

                mo = mp.tile([P, P], FP, tag="mself")
                nc.sync.dma_start(mo[:, :], own_dram[b * P : (b + 1) * P, :])
                s = s_tile(int(self_chunk[b]))
                nc.tensor.matmul(
                    out=ps[:, :], lhsT=mo[:, :], rhs=s[:, :],
                    start=True, stop=(nchunks == 1),
                )
                ci = 1
                for j in range(NBANK):
                    if Kbj[b][j] == 0:
                        continue
                    loc0 = int(cumK[j, b] - cumK[j, blocks[0]])
                    for k in range(Kbj[b][j]):
                        ch = int(chunk_base[b, j]) + k
                        s = s_tile(ch)
                        nc.tensor.matmul(
                            out=ps[:, :],
                            lhsT=mts[j][:, loc0 + k, :],
                            rhs=s[:, :],
                            start=False,
                            stop=(ci == nchunks - 1),
                        )
                        ci += 1
                return ps

            # ---------------- layer 1 + dense transform to t2 ----------------
            for g in range(ngroups):
                blocks = list(range(g * GB, min((g + 1) * GB, n_blocks)))
                nb = len(blocks)
                W = nb * P
                mts = gather_group(x_d, n_x_rows, blocks)
                aggsb = aggs.tile([P, GB * P], FP, tag="aggsb")
                for i, b in enumerate(blocks):
                    ps = spmm_block(mts, blocks, b, xo_d)
                    nc.scalar.activation(aggsb[:, i * P : (i + 1) * P], ps[:, :], AF.Copy)
                h1 = []
                for h in range(2):
                    hp = dps.tile([P, GB * P], FP, tag="big")
                    nc.tensor.matmul(
                        out=hp[:, :W],
                        lhsT=(w1a, w1b)[h][:, :],
                        rhs=aggsb[:, :W],
                        start=True,
                        stop=True,
                    )
                    hb = hs.tile([P, GB * P], FP, tag=f"h1{h}")
                    nc.scalar.activation(hb[:, :W], hp[:, :W], AF.Relu, bias=b1t[:, h : h + 1])
                    h1.append(hb)
                tp_ = dps.tile([P, GB * P], FP, tag="big")
                nc.tensor.matmul(out=tp_[:, :W], lhsT=w2a[:, :], rhs=h1[0][:, :W], start=True, stop=False)
                nc.tensor.matmul(out=tp_[:, :W], lhsT=w2b[:, :], rhs=h1[1][:, :W], start=False, stop=True)
                t2b = t2s.tile([P, GB * P], FP, tag="t2b")
                nc.scalar.activation(t2b[:, :W], tp_[:, :W], AF.Copy)
                for i, b in enumerate(blocks):
                    tpps = tpp.tile([P, P], FP, tag="tp")
                    nc.tensor.transpose(out=tpps[:, :], in_=t2b[:, i * P : (i + 1) * P], identity=ident[:, :])
                    tsb = tps.tile([P, P], FP, tag="tsb")
                    nc.scalar.activation(tsb[:, :], tpps[:, :], AF.Copy)
                    r0 = b * P
                    nc.sync.dma_start(t2_own[r0 : r0 + P, :], tsb[:, :])

            # ---------------- exchange t2 shards ----------------
            if timing_variant:
                # single-core timing build: stand-in DMA for the collective
                # (its real cost is added from the measured-latency table)
                nc.sync.dma_start(t2_cat[0:OWN, :], t2_own[:, :])
            else:
                nc.gpsimd.collective_compute(
                    "AllGather",
                    ALU.bypass,
                    replica_groups=[list(range(NCORES))],
                    ins=[t2_own[:, :]],
                    outs=[t2_cat[:, :]],
                )

            # ---------------- layer 2 ----------------
            for g in range(ngroups):
                blocks = list(range(g * GB, min((g + 1) * GB, n_blocks)))
                mts = gather_group(t2_cat, n_cat_rows, blocks)
                for b in blocks:
                    ps = spmm_block(mts, blocks, b, t2_own)
                    z = zs.tile([P, P], FP, tag="z")
                    nc.scalar.activation(z[:, :], ps[:, :], AF.Relu, bias=b2t[:, 0:1])
                    tpps = tpp.tile([P, P], FP, tag="tp")
                    nc.tensor.transpose(out=tpps[:, :], in_=z[:, :], identity=ident[:, :])
                    tsb = tps.tile([P, P], FP, tag="tsb")
                    nc.scalar.activation(tsb[:, :], tpps[:, :], AF.Copy)
                    r0 = b * P
                    nc.sync.dma_start(out_d[r0 : r0 + P, :], tsb[:, :])

    nc.compile()
    return nc


def _preprocess(x, edge_index, W1, b1, W2, b2):
    N = x.shape[0]
    OWN = int(math.ceil(N / (NCORES * P))) * P
    n_blocks = OWN // P
    NBLK = NCORES * n_blocks

    src = np.asarray(edge_index[0], np.int64)
    dst = np.asarray(edge_index[1], np.int64)

    # degree includes the self-loop; self-loops are handled as dense per-block
    # chunks (S = diag(dinv^2)) rather than gathered edges.
    deg = (np.bincount(dst, minlength=N) + 1).astype(np.float64)
    dinv = (1.0 / np.sqrt(deg)).astype(np.float32)
    norm_e = dinv[src] * dinv[dst]

    gblk = dst // P
    bank = src // BANK
    cell = gblk * NBANK + bank
    order = np.argsort(cell, kind="stable")
    s_src = src[order]
    s_dst = dst[order]
    s_norm = norm_e[order].astype(np.float32)
    s_cell = cell[order]
    s_bank = s_src // BANK

    counts = np.bincount(s_cell, minlength=NBLK * NBANK)
    percell = counts.reshape(NCORES, n_blocks, NBANK)
    Kbj = np.ceil(percell.max(axis=0) / P).astype(np.int64)  # [n_blocks, NBANK]
    caps = Kbj * P

    # slot offsets within the per-core padded banked-edge stream
    cell_off = np.concatenate(([0], np.cumsum(caps.ravel())))[:-1].reshape(n_blocks, NBANK)
    TOT = int(caps.sum())
    CHB = int(Kbj.sum())  # banked chunks
    CH = n_blocks + CHB  # + one self chunk per block

    starts = np.concatenate(([0], np.cumsum(counts)))[:-1]
    pos = np.arange(s_dst.size) - starts[s_cell]
    core = (gblk[order] // n_blocks).astype(np.int64)
    lblk = (gblk[order] % n_blocks).astype(np.int64)
    slot = cell_off[lblk, s_bank] + pos

    arr_rel = np.zeros((NCORES, TOT), np.int16)
    arr_dst = np.zeros((NCORES, TOT), np.float32)
    arr_nrm = np.zeros((NCORES, TOT), np.float32)
    arr_rel[core, slot] = (s_src - s_bank * BANK).astype(np.int16)
    arr_dst[core, slot] = (s_dst % P).astype(np.float32)
    arr_nrm[core, slot] = s_norm

    # global chunk order per block: [self, bank chunks...]
    bdst3 = arr_dst.reshape(NCORES, CHB, P)
    bnrm3 = arr_nrm.reshape(NCORES, CHB, P)
    stage_dst = np.zeros((NCORES, CH, P), np.float32)
    stage_nrm = np.zeros((NCORES, CH, P), np.float32)
    # self-chunk values
    dinv2 = np.zeros(NCORES * OWN, np.float32)
    dinv2[:N] = dinv * dinv
    dinv2 = dinv2.reshape(NCORES, n_blocks, P)
    is_self = np.zeros(CH, bool)
    chunk_of_banked = np.zeros(CHB, np.int64)
    acc = 0
    bi = 0
    for b in range(n_blocks):
        is_self[acc] = True
        stage_dst[:, acc, :] = np.arange(P, dtype=np.float32)[None, :]
        stage_nrm[:, acc, :] = dinv2[:, b, :]
        acc += 1
        nb = int(Kbj[b].sum())
        chunk_of_banked[bi : bi + nb] = np.arange(acc, acc + nb)
        acc += nb
        bi += nb
    stage_dst[:, chunk_of_banked, :] = bdst3
    stage_nrm[:, chunk_of_banked, :] = bnrm3
    stage_dst = np.ascontiguousarray(stage_dst.transpose(0, 2, 1))
    stage_nrm = np.ascontiguousarray(stage_nrm.transpose(0, 2, 1))

    # per-bank int16 index streams, 16-partition wrapped, replicated to 128 rows
    chunk_bank = np.repeat(
        np.tile(np.arange(NBANK), n_blocks), Kbj.ravel()
    )  # [CHB] bank of each banked chunk
    rel3 = arr_rel.reshape(NCORES, CHB, P)
    idx_stages = []
    for j in range(NBANK):
        selj = chunk_bank == j
        cj = int(selj.sum())
        if cj == 0:
            idx_stages.append(np.zeros((NCORES, P, 8), np.int16))
            continue
        flat = rel3[:, selj, :].reshape(NCORES, cj * P)
        w = flat.reshape(NCORES, cj * 8, 16).transpose(0, 2, 1)  # [cores, 16, cj*8]
        idx_stages.append(np.ascontiguousarray(np.tile(w, (1, 8, 1))))

    # per-core own-shard rows (source of the self chunks), zero-padded
    xo = np.zeros((NCORES * OWN, D_IN), np.float32)
    xo[:N] = np.asarray(x, np.float32)
    xo = np.ascontiguousarray(xo.reshape(NCORES, OWN, D_IN))

    xf = np.ascontiguousarray(np.asarray(x, np.float32))
    w1 = np.ascontiguousarray(np.asarray(W1, np.float32))
    w2 = np.ascontiguousarray(np.asarray(W2, np.float32))
    b1h = np.ascontiguousarray(np.asarray(b1, np.float32).reshape(2, P).T)
    b2c = np.ascontiguousarray(np.asarray(b2, np.float32).reshape(P, 1))
    iota = np.ascontiguousarray(np.tile(np.arange(P, dtype=np.float32), (P, 1)))

    in_maps = []
    for c in range(NCORES):
        m = {
            "x": xf,
            "x_own": xo[c],
            "w1": w1,
            "w2": w2,
            "b1h": b1h,
            "b2c": b2c,
            "iota": iota,
            "dst_rel": stage_dst[c],
            "norm": stage_nrm[c],
        }
        for j in range(NBANK):
            m[f"idx{j}"] = idx_stages[j][c]
        in_maps.append(m)
    return in_maps, N, OWN, n_blocks, [list(map(int, r)) for r in Kbj], CH


def run(x, edge_index, W1, b1, W2, b2, trace=False):
    from concourse.bass_utils import run_bass_kernel_spmd

    in_maps, N, OWN, n_blocks, Kbj, CH = _preprocess(x, edge_index, W1, b1, W2, b2)
    key = (N, OWN, n_blocks, CH, tuple(tuple(r) for r in Kbj))
    nc = _CACHE.get(key)
    if nc is None:
        nc = _build(N, OWN, n_blocks, Kbj, CH)
        _CACHE[key] = nc

    res = run_bass_kernel_spmd(nc, in_maps, core_ids=list(range(NCORES)), trace=trace)
    out = np.concatenate([res.results[c]["out"] for c in range(NCORES)], axis=0)[:N]
    return np.ascontiguousarray(out.astype(np.float32)), res


def kernel(x, edge_index, W1, b1, W2, b2):
    out, _ = run(x, edge_index, W1, b1, W2, b2, trace=False)
    return out


def estimate_time_ns(np_inputs):
    """Cost-model (TimelineSim) per-core time estimate + AllGather table cost."""
    from concourse.timeline_sim import TimelineSim

    in_maps, N, OWN, n_blocks, Kbj, CH = _preprocess(**np_inputs)
    key = ("timing", N, OWN, n_blocks, CH, tuple(tuple(r) for r in Kbj))
    nc = _CACHE.get(key)
    if nc is None:
        nc = _build(N, OWN, n_blocks, Kbj, CH, timing_variant=True)
        _CACHE[key] = nc
    ts = TimelineSim(nc)
    t = ts.simulate()
    AG_NS = 35000.0  # 8-core AllGather @ ~6.4MB/rank (measured-latency table)
    return t + AG_NS


# revision 20
# speedup vs baseline: 1.5348x; 1.3079x over previous
"""2-layer GCN (GCNConv -> ReLU -> GCNConv -> ReLU) on 8 Trainium2 NeuronCores.

Math (per layer, following PyG GCNConv):
    out = D^-1/2 (A + I) D^-1/2 (x @ W) + b
We exploit associativity so the sparse aggregation always runs on 128 features:
    layer1: h1 = relu( (A_norm @ x) @ W1 + b1 )          (aggregate first)
    layer2: out = relu( A_norm @ (h1 @ W2) + b2 )        (transform first)
Self-loops are appended as ordinary edges; per-edge weight norm_e =
dinv[src]*dinv[dst] makes the weighted segment-sum exactly A_norm.

Sharding: nodes are split into 8 contiguous shards of OWN=ceil(N/1024)*128
rows; edges are partitioned by destination owner so each core's segment-sum
is local.  Each core gathers source rows from a full replica of x (layer 1)
and from an AllGather'ed t2 = h1@W2 (layer 2).

Gathers use the gpsimd dma_gather custom op (one instruction fetches
thousands of rows).  Its indices are int16, so the source table is viewed in
4 banks of 25088 rows; edges are bucketed per (dst-block, src-bank), each
bucket padded to a multiple of 128 with dummy index 0 / weight 0.

Device algorithm per 128-node destination block (chunks of 128 edges):
    - per-bank dma_gather fetches the chunk rows   -> M [128e, C, 128f]
    - DVE builds S^T[e, i] = (iota[i] == dst_rel[e]) * norm[e] per chunk
    - PE accumulates  psum[f, i] += M_c^T @ S_c^T  over the block's chunks
giving the aggregated block transposed ([feat, dst]), which feeds the dense
transforms without any transpose; PE transpose mode is used only to emit
row-major t2 / output tiles.
"""

import math

import ml_dtypes
import numpy as np

P = 128
NCORES = 8
D_IN, D_HID, D_OUT = 128, 256, 128
GB = 4  # dst blocks per dense group (psum free dim = GB*128 <= 512)
BANK = 25088  # int16-addressable rows per gather-table view
NBANK = 4

_CACHE: dict = {}


def _build(n_x_rows, OWN, n_blocks, Kbj, CH, timing_variant=False):
    import concourse.bacc as bacc
    import concourse.mybir as mybir
    import concourse.tile as tile
    from concourse.masks import make_identity

    FP = mybir.dt.float32
    BF = mybir.dt.bfloat16
    R32 = mybir.dt.float32r
    I16 = mybir.dt.int16
    AF = mybir.ActivationFunctionType
    ALU = mybir.AluOpType

    n_cat_rows = OWN * NCORES
    # chunk bookkeeping (shared across cores)
    # per block: [self chunk, bank0 chunks..., bank3 chunks...]
    self_chunk = np.zeros(n_blocks, np.int64)
    chunk_base = np.zeros((n_blocks, NBANK), np.int64)
    acc = 0
    for b in range(n_blocks):
        self_chunk[b] = acc
        acc += 1
        for j in range(NBANK):
            chunk_base[b, j] = acc
            acc += Kbj[b][j]
    assert acc == CH
    # per-bank cumulative chunk counts (for idx-column offsets)
    cumK = np.zeros((NBANK, n_blocks + 1), np.int64)
    for j in range(NBANK):
        for b in range(n_blocks):
            cumK[j, b + 1] = cumK[j, b] + Kbj[b][j]
    Cj = [int(cumK[j, n_blocks]) for j in range(NBANK)]

    NSWQ = 4  # rotate gathers over SWDGE queues
    MAXC = 8  # chunks per dma_gather call (1024 descs = SWDGE ring capacity)
    nc = bacc.Bacc("TRN2", debug=False, num_devices=NCORES, num_swdge_queues=NSWQ)

    x_d = nc.dram_tensor("x", [n_x_rows, D_IN], BF, kind="ExternalInput")
    xo_d = nc.dram_tensor("x_own", [OWN, D_IN], BF, kind="ExternalInput")
    w1_d = nc.dram_tensor("w1", [D_IN, D_HID], FP, kind="ExternalInput")
    w2_d = nc.dram_tensor("w2", [D_HID, D_OUT], FP, kind="ExternalInput")
    b1_d = nc.dram_tensor("b1h", [P, 2], FP, kind="ExternalInput")
    b2_d = nc.dram_tensor("b2c", [P, 1], FP, kind="ExternalInput")
    iota_d = nc.dram_tensor("iota", [P, P], BF, kind="ExternalInput")
    identb_d = nc.dram_tensor("identb", [P, P], BF, kind="ExternalInput")
    dr_d = nc.dram_tensor("dst_rel", [P, CH], FP, kind="ExternalInput")
    nm_d = nc.dram_tensor("norm", [P, CH], FP, kind="ExternalInput")
    idx_ds = [
        nc.dram_tensor(f"idx{j}", [P, max(Cj[j], 1) * 8], I16, kind="ExternalInput")
        for j in range(NBANK)
    ]
    out_d = nc.dram_tensor("out", [OWN, D_OUT], FP, kind="ExternalOutput")
    t2_own = nc.dram_tensor("t2_own", [OWN, D_OUT], BF)
    t2_cat = nc.dram_tensor("t2_cat", [n_cat_rows, D_OUT], BF, addr_space="Shared")

    ngroups = (n_blocks + GB - 1) // GB

    with tile.TileContext(nc) as tc:
        with (
            tc.tile_pool(name="const", bufs=1) as constp,
            tc.tile_pool(name="mp", bufs=3) as mp,
            tc.tile_pool(name="sp", bufs=6) as sp,
            tc.tile_pool(name="aggs", bufs=2) as aggs,
            tc.tile_pool(name="hs", bufs=2) as hs,
            tc.tile_pool(name="t2s", bufs=2) as t2s,
            tc.tile_pool(name="tps", bufs=3) as tps,
            tc.tile_pool(name="zs", bufs=2) as zs,
            tc.tile_pool(name="aggp", bufs=2, space="PSUM") as aggp,
            tc.tile_pool(name="dps", bufs=2, space="PSUM") as dps,
            tc.tile_pool(name="tpp", bufs=2, space="PSUM") as tpp,
        ):
            iota_t = constp.tile([P, P], BF, tag="iota")
            nc.sync.dma_start(iota_t[:, :], iota_d[:, :])
            ident = constp.tile([P, P], FP, tag="ident")
            make_identity(nc, ident[:, :])
            identb = constp.tile([P, P], BF, tag="identb")
            nc.sync.dma_start(identb[:, :], identb_d[:, :])
            w1a_f = constp.tile([P, P], FP, tag="w1a_f")
            nc.sync.dma_start(w1a_f[:, :], w1_d[:, 0:P])
            w1b_f = constp.tile([P, P], FP, tag="w1b_f")
            nc.sync.dma_start(w1b_f[:, :], w1_d[:, P : 2 * P])
            w2a_f = constp.tile([P, P], FP, tag="w2a_f")
            nc.sync.dma_start(w2a_f[:, :], w2_d[0:P, :])
            w2b_f = constp.tile([P, P], FP, tag="w2b_f")
            nc.sync.dma_start(w2b_f[:, :], w2_d[P : 2 * P, :])
            w1a = constp.tile([P, P], R32, tag="w1a")
            nc.vector.tensor_copy(w1a[:, :], w1a_f[:, :])
            w1b = constp.tile([P, P], R32, tag="w1b")
            nc.vector.tensor_copy(w1b[:, :], w1b_f[:, :])
            w2a = constp.tile([P, P], R32, tag="w2a")
            nc.vector.tensor_copy(w2a[:, :], w2a_f[:, :])
            w2b = constp.tile([P, P], R32, tag="w2b")
            nc.vector.tensor_copy(w2b[:, :], w2b_f[:, :])
            b1t = constp.tile([P, 2], FP, tag="b1")
            nc.sync.dma_start(b1t[:, :], b1_d[:, :])
            b2t = constp.tile([P, 1], FP, tag="b2")
            nc.sync.dma_start(b2t[:, :], b2_d[:, :])
            sdst = constp.tile([P, CH], FP, tag="sdst")
            nc.sync.dma_start(sdst[:, :], dr_d[:, :])
            snorm = constp.tile([P, CH], FP, tag="snorm")
            nc.sync.dma_start(snorm[:, :], nm_d[:, :])
            idx_ts = []
            for j in range(NBANK):
                it = constp.tile([P, max(Cj[j], 1) * 8], I16, tag=f"idx{j}", name=f"idxt{j}")
                nc.sync.dma_start(it[:, :], idx_ds[j][:, :])
                idx_ts.append(it)

            qrot = [0]

            def gather_group(src_dram, n_rows, blocks):
                """dma_gather calls (<= MAXC chunks each) per source bank."""
                mts = {}
                for j in range(NBANK):
                    cg = int(cumK[j, blocks[-1] + 1] - cumK[j, blocks[0]])
                    if cg == 0:
                        continue
                    mt = mp.tile([P, cg, P], BF, tag=f"m{j}", name=f"mt{j}")
                    s0 = int(cumK[j, blocks[0]]) * 8
                    lo = j * BANK
                    hi = min((j + 1) * BANK, n_rows)
                    for c0 in range(0, cg, MAXC):
                        cc = min(MAXC, cg - c0)
                        nc.gpsimd.dma_gather(
                            out_ap=mt[:, c0 : c0 + cc, :],
                            in_ap=src_dram[lo:hi, :],
                            idxs_ap=idx_ts[j][:, s0 + c0 * 8 : s0 + (c0 + cc) * 8],
                            num_idxs=cc * P,
                            num_idxs_reg=cc * P,
                            elem_size=P,
                            queue_num=qrot[0] % NSWQ,
                        )
                        qrot[0] += 1
                    mts[j] = mt
                return mts

            def spmm_block(mts, blocks, b, own_dram):
                """Weighted segment-sum of block b -> PSUM [128 feat, 128 dst].

                Chunk 0 is the self-loop chunk: its "gathered" rows are the
                block's own contiguous rows (plain DMA), S = diag(dinv^2)."""
                ps = aggp.tile([P, P], FP, tag="agg")
                nchunks = 1 + sum(Kbj[b])

                def s_tile(ch):
                    s = sp.tile([P, P], BF, tag="s", name="s")
                    nc.vector.tensor_scalar(
                        out=s[:, :],
                        in0=iota_t[:, :],
                        scalar1=sdst[:, ch : ch + 1],
                        scalar2=snorm[:, ch : ch + 1],
                        op0=ALU.is_equal,
                        op1=ALU.mult,
                    )
                    return s

                mo = mp.tile([P, P], BF, tag="mself")
                nc.sync.dma_start(mo[:, :], own_dram[b * P : (b + 1) * P, :])
                s = s_tile(int(self_chunk[b]))
                nc.tensor.matmul(
                    out=ps[:, :], lhsT=mo[:, :], rhs=s[:, :],
                    start=True, stop=(nchunks == 1),
                )
                ci = 1
                for j in range(NBANK):
                    if Kbj[b][j] == 0:
                        continue
                    loc0 = int(cumK[j, b] - cumK[j, blocks[0]])
                    for k in range(Kbj[b][j]):
                        ch = int(chunk_base[b, j]) + k
                        s = s_tile(ch)
                        nc.tensor.matmul(
                            out=ps[:, :],
                            lhsT=mts[j][:, loc0 + k, :],
                            rhs=s[:, :],
                            start=False,
                            stop=(ci == nchunks - 1),
                        )
                        ci += 1
                return ps

            # ---------------- layer 1 + dense transform to t2 ----------------
            for g in range(ngroups):
                blocks = list(range(g * GB, min((g + 1) * GB, n_blocks)))
                nb = len(blocks)
                W = nb * P
                mts = gather_group(x_d, n_x_rows, blocks)
                aggsb = aggs.tile([P, GB * P], R32, tag="aggsb")
                for i, b in enumerate(blocks):
                    ps = spmm_block(mts, blocks, b, xo_d)
                    nc.scalar.activation(aggsb[:, i * P : (i + 1) * P], ps[:, :], AF.Copy)
                h1 = []
                for h in range(2):
                    hp = dps.tile([P, GB * P], FP, tag="big")
                    nc.tensor.matmul(
                        out=hp[:, :W],
                        lhsT=(w1a, w1b)[h][:, :],
                        rhs=aggsb[:, :W],
                        start=True,
                        stop=True,
                    )
                    hb = hs.tile([P, GB * P], R32, tag=f"h1{h}")
                    nc.scalar.activation(hb[:, :W], hp[:, :W], AF.Relu, bias=b1t[:, h : h + 1])
                    h1.append(hb)
                tp_ = dps.tile([P, GB * P], FP, tag="big")
                nc.tensor.matmul(out=tp_[:, :W], lhsT=w2a[:, :], rhs=h1[0][:, :W], start=True, stop=False)
                nc.tensor.matmul(out=tp_[:, :W], lhsT=w2b[:, :], rhs=h1[1][:, :W], start=False, stop=True)
                t2b = t2s.tile([P, GB * P], BF, tag="t2b")
                nc.scalar.activation(t2b[:, :W], tp_[:, :W], AF.Copy)
                for i, b in enumerate(blocks):
                    tpps = tpp.tile([P, P], BF, tag="tp_b", bufs=2)
                    nc.tensor.transpose(out=tpps[:, :], in_=t2b[:, i * P : (i + 1) * P], identity=identb[:, :])
                    tsb = tps.tile([P, P], BF, tag="tsb_b")
                    nc.scalar.activation(tsb[:, :], tpps[:, :], AF.Copy)
                    r0 = b * P
                    nc.sync.dma_start(t2_own[r0 : r0 + P, :], tsb[:, :])

            # ---------------- exchange t2 shards ----------------
            if timing_variant:
                # single-core timing build: stand-in DMA for the collective
                # (its real cost is added from the measured-latency table)
                nc.sync.dma_start(t2_cat[0:OWN, :], t2_own[:, :])
            else:
                nc.gpsimd.collective_compute(
                    "AllGather",
                    ALU.bypass,
                    replica_groups=[list(range(NCORES))],
                    ins=[t2_own[:, :]],
                    outs=[t2_cat[:, :]],
                )

            # ---------------- layer 2 ----------------
            for g in range(ngroups):
                blocks = list(range(g * GB, min((g + 1) * GB, n_blocks)))
                mts = gather_group(t2_cat, n_cat_rows, blocks)
                for b in blocks:
                    ps = spmm_block(mts, blocks, b, t2_own)
                    z = zs.tile([P, P], FP, tag="z")
                    nc.scalar.activation(z[:, :], ps[:, :], AF.Relu, bias=b2t[:, 0:1])
                    tpps = tpp.tile([P, P], FP, tag="tp")
                    nc.tensor.transpose(out=tpps[:, :], in_=z[:, :], identity=ident[:, :])
                    tsb = tps.tile([P, P], FP, tag="tsb_f")
                    nc.scalar.activation(tsb[:, :], tpps[:, :], AF.Copy)
                    r0 = b * P
                    nc.sync.dma_start(out_d[r0 : r0 + P, :], tsb[:, :])

    nc.compile()
    return nc


def _preprocess(x, edge_index, W1, b1, W2, b2):
    N = x.shape[0]
    OWN = int(math.ceil(N / (NCORES * P))) * P
    n_blocks = OWN // P
    NBLK = NCORES * n_blocks

    src = np.asarray(edge_index[0], np.int64)
    dst = np.asarray(edge_index[1], np.int64)

    # degree includes the self-loop; self-loops are handled as dense per-block
    # chunks (S = diag(dinv^2)) rather than gathered edges.
    deg = (np.bincount(dst, minlength=N) + 1).astype(np.float64)
    dinv = (1.0 / np.sqrt(deg)).astype(np.float32)
    norm_e = dinv[src] * dinv[dst]

    gblk = dst // P
    bank = src // BANK
    cell = gblk * NBANK + bank
    order = np.argsort(cell, kind="stable")
    s_src = src[order]
    s_dst = dst[order]
    s_norm = norm_e[order].astype(np.float32)
    s_cell = cell[order]
    s_bank = s_src // BANK

    counts = np.bincount(s_cell, minlength=NBLK * NBANK)
    percell = counts.reshape(NCORES, n_blocks, NBANK)
    Kbj = np.ceil(percell.max(axis=0) / P).astype(np.int64)  # [n_blocks, NBANK]
    caps = Kbj * P

    # slot offsets within the per-core padded banked-edge stream
    cell_off = np.concatenate(([0], np.cumsum(caps.ravel())))[:-1].reshape(n_blocks, NBANK)
    TOT = int(caps.sum())
    CHB = int(Kbj.sum())  # banked chunks
    CH = n_blocks + CHB  # + one self chunk per block

    starts = np.concatenate(([0], np.cumsum(counts)))[:-1]
    pos = np.arange(s_dst.size) - starts[s_cell]
    core = (gblk[order] // n_blocks).astype(np.int64)
    lblk = (gblk[order] % n_blocks).astype(np.int64)
    slot = cell_off[lblk, s_bank] + pos

    arr_rel = np.zeros((NCORES, TOT), np.int16)
    arr_dst = np.zeros((NCORES, TOT), np.float32)
    arr_nrm = np.zeros((NCORES, TOT), np.float32)
    arr_rel[core, slot] = (s_src - s_bank * BANK).astype(np.int16)
    arr_dst[core, slot] = (s_dst % P).astype(np.float32)
    arr_nrm[core, slot] = s_norm

    # global chunk order per block: [self, bank chunks...]
    bdst3 = arr_dst.reshape(NCORES, CHB, P)
    bnrm3 = arr_nrm.reshape(NCORES, CHB, P)
    stage_dst = np.zeros((NCORES, CH, P), np.float32)
    stage_nrm = np.zeros((NCORES, CH, P), np.float32)
    # self-chunk values
    dinv2 = np.zeros(NCORES * OWN, np.float32)
    dinv2[:N] = dinv * dinv
    dinv2 = dinv2.reshape(NCORES, n_blocks, P)
    is_self = np.zeros(CH, bool)
    chunk_of_banked = np.zeros(CHB, np.int64)
    acc = 0
    bi = 0
    for b in range(n_blocks):
        is_self[acc] = True
        stage_dst[:, acc, :] = np.arange(P, dtype=np.float32)[None, :]
        stage_nrm[:, acc, :] = dinv2[:, b, :]
        acc += 1
        nb = int(Kbj[b].sum())
        chunk_of_banked[bi : bi + nb] = np.arange(acc, acc + nb)
        acc += nb
        bi += nb
    stage_dst[:, chunk_of_banked, :] = bdst3
    stage_nrm[:, chunk_of_banked, :] = bnrm3
    BFNP = ml_dtypes.bfloat16
    stage_dst = np.ascontiguousarray(stage_dst.transpose(0, 2, 1))
    stage_nrm = np.ascontiguousarray(stage_nrm.transpose(0, 2, 1))

    # per-bank int16 index streams, 16-partition wrapped, replicated to 128 rows
    chunk_bank = np.repeat(
        np.tile(np.arange(NBANK), n_blocks), Kbj.ravel()
    )  # [CHB] bank of each banked chunk
    rel3 = arr_rel.reshape(NCORES, CHB, P)
    idx_stages = []
    for j in range(NBANK):
        selj = chunk_bank == j
        cj = int(selj.sum())
        if cj == 0:
            idx_stages.append(np.zeros((NCORES, P, 8), np.int16))
            continue
        flat = rel3[:, selj, :].reshape(NCORES, cj * P)
        w = flat.reshape(NCORES, cj * 8, 16).transpose(0, 2, 1)  # [cores, 16, cj*8]
        idx_stages.append(np.ascontiguousarray(np.tile(w, (1, 8, 1))))

    # per-core own-shard rows (source of the self chunks), zero-padded
    xo = np.zeros((NCORES * OWN, D_IN), BFNP)
    xo[:N] = np.asarray(x, np.float32).astype(BFNP)
    xo = np.ascontiguousarray(xo.reshape(NCORES, OWN, D_IN))

    xf = np.ascontiguousarray(np.asarray(x, np.float32).astype(BFNP))
    w1 = np.ascontiguousarray(np.asarray(W1, np.float32))
    w2 = np.ascontiguousarray(np.asarray(W2, np.float32))
    b1h = np.ascontiguousarray(np.asarray(b1, np.float32).reshape(2, P).T)
    b2c = np.ascontiguousarray(np.asarray(b2, np.float32).reshape(P, 1))
    iota = np.ascontiguousarray(np.tile(np.arange(P), (P, 1)).astype(BFNP))
    identb = np.ascontiguousarray(np.eye(P).astype(BFNP))

    in_maps = []
    for c in range(NCORES):
        m = {
            "x": xf,
            "x_own": xo[c],
            "w1": w1,
            "w2": w2,
            "b1h": b1h,
            "b2c": b2c,
            "iota": iota,
            "identb": identb,
            "dst_rel": stage_dst[c],
            "norm": stage_nrm[c],
        }
        for j in range(NBANK):
            m[f"idx{j}"] = idx_stages[j][c]
        in_maps.append(m)
    return in_maps, N, OWN, n_blocks, [list(map(int, r)) for r in Kbj], CH


def run(x, edge_index, W1, b1, W2, b2, trace=False):
    from concourse.bass_utils import run_bass_kernel_spmd

    in_maps, N, OWN, n_blocks, Kbj, CH = _preprocess(x, edge_index, W1, b1, W2, b2)
    key = (N, OWN, n_blocks, CH, tuple(tuple(r) for r in Kbj))
    nc = _CACHE.get(key)
    if nc is None:
        nc = _build(N, OWN, n_blocks, Kbj, CH)
        _CACHE[key] = nc

    res = run_bass_kernel_spmd(nc, in_maps, core_ids=list(range(NCORES)), trace=trace)
    out = np.concatenate([res.results[c]["out"] for c in range(NCORES)], axis=0)[:N]
    return np.ascontiguousarray(out.astype(np.float32)), res


def kernel(x, edge_index, W1, b1, W2, b2):
    out, _ = run(x, edge_index, W1, b1, W2, b2, trace=False)
    return out


def estimate_time_ns(np_inputs):
    """Cost-model (TimelineSim) per-core time estimate + AllGather table cost."""
    from concourse.timeline_sim import TimelineSim

    in_maps, N, OWN, n_blocks, Kbj, CH = _preprocess(**np_inputs)
    key = ("timing", N, OWN, n_blocks, CH, tuple(tuple(r) for r in Kbj))
    nc = _CACHE.get(key)
    if nc is None:
        nc = _build(N, OWN, n_blocks, Kbj, CH, timing_variant=True)
        _CACHE[key] = nc
    ts = TimelineSim(nc)
    t = ts.simulate()
    AG_NS = 35000.0  # 8-core AllGather @ ~6.4MB/rank (measured-latency table)
    return t + AG_NS
